# revision 41
# baseline (speedup 1.0000x reference)
"""Gaussian resampling kernel for Trainium2 (8 NeuronCores, SPMD).

Computes, for each batch row b:
    e = cumsum(d); c = e - d/2
    w[t, s] = softmax_s(-(t - c_s)^2 / 10)   (masked s get weight 0)
    out[t, :] = sum_s w[t, s] * x[s, :]

Strategy:
  - Host precomputes c (float64 cumsum); data-parallel over batch:
    2 batches per core on 8 cores.
  - Scores are built in [S, T] layout (tokens on partitions) in ONE ACT
    pass: Derivative_Erf(x) = (2/sqrt(pi)) * exp(-x^2). The hardware
    table was probed at ~1.3e-5 relative accuracy with exact zero
    tails. Masked tokens' x rows are zeroed on the host so their
    scores are inert regardless of table tail behavior; their centers
    are moved to -3000 to keep the table input bounded.
  - The softmax denominator D_t = sum_s exp(-(t-c_s)^2/10) depends only
    on the duration vector d, so the host precomputes rcl = sqrt(pi)/2
    / D (the 2/sqrt(pi) from the table folds in) and ships it as a
    [P, MC] per-batch constant. The device then never computes
    denominators or reciprocals: normalization is a single
    per-partition scalar multiply per output chunk, split between ACT
    and DVE to balance their load. Output is written bf16 (rel rounding
    2^-9, far inside the 2e-2 gate) and cast to f32 on the host.
  - Banded sparsity: centers are monotone, so each 128-token chunk only
    has non-negligible scores in a contiguous frame range. The bands
    (unioned over the batches in a slot, so the SPMD program is shared)
    are computed on the host and baked into the program; batches are
    sorted by valid length and paired into per-core slots of similar
    length to keep slot band unions tight.
  - Output DMAs are grouped 4 frame-chunks at a time to amortize the
    ~0.7us per-DMA sequencer issue cost. Matmuls in fp16; junk matmuls
    at startup warm the PE clock gate, and frame indices come from
    GpSimd iota to keep the DMA wire free for real input traffic.
"""

import math
import sys
import types

import numpy as np

# ---------------------------------------------------------------------------
# Optional NTFF-profiling plumbing. The runtime image lacks
# antenv.axon_hooks; wire a stand-in so run_bass_kernel_spmd(trace=True)
# works (used by the dev harness; the plain kernel path never traces).
try:  # pragma: no cover - best effort
    import antenv.axon_hooks  # noqa: F401
except ImportError:
    try:
        _hooks_mod = types.ModuleType("antenv.axon_hooks")
        _hook_box = [None]
        _hooks_mod.set_axon_ntff_profile_hook = (
            lambda hook: _hook_box.__setitem__(0, hook)
        )
        _hooks_mod.get_axon_ntff_profile_hook = lambda: _hook_box[0]
        sys.modules["antenv.axon_hooks"] = _hooks_mod
        from trn_agent_boot.trn_boot import _ntff_profile_via_ctypes

        _hooks_mod.set_axon_ntff_profile_hook(
            _ntff_profile_via_ctypes("/opt/axon/libaxon_pjrt.so")
        )
    except Exception:
        pass

import concourse.bacc as bacc
import concourse.mybir as mybir
import concourse.tile as tile
import concourse.bass_utils as bass_utils
from concourse.tile_rust import add_dep_helper

# Avoid S3 artifact uploads from the trace path in this container.
bass_utils.upload_artifacts = lambda tmpdir: f"local:{tmpdir}"

from concourse.bass_utils import run_bass_kernel_spmd

NCORES = 8
B, S, D, T = 16, 512, 768, 4096
VARIANCE = 10.0
BPC = B // NCORES          # batches per core
P = 128                    # partitions
KC = S // P                # token chunks (4)
MC = T // P                # output frame chunks (32)
N0 = 512                   # first matmul column split (one PSUM bank)
MARGIN = 16.0              # frames; fp16 scores flush below ~13 anyway
ACT_PIECE = 2048           # max free-dim length of one score ACT op

_PROGRAMS = {}


def _compute_bands(c_masked):
    """Per token-chunk [lo, hi) active frame range (128-aligned), unioned
    over the given batches. c_masked: (n, S) float64, masked tokens nan.
    A fully-masked chunk yields None (skipped entirely)."""
    bands = []
    for k in range(KC):
        ck = c_masked[:, k * P:(k + 1) * P]
        if np.all(np.isnan(ck)):
            bands.append(None)
            continue
        lo = np.nanmin(ck) - MARGIN
        hi = np.nanmax(ck) + MARGIN
        a = max(0, int(math.floor(lo - 1)) // P * P)
        b = min(T, -(-int(math.ceil(hi)) // P) * P)
        b = max(b, a + P)
        bands.append((a, b))
    return tuple(bands)


def _build_program(bands2):
    """bands2: per batch-slot tuple of per-chunk (a, b) bands (or None)."""
    nc = bacc.Bacc("TRN2", target_bir_lowering=False, debug=False)
    f32 = mybir.dt.float32
    fp16 = mybir.dt.float16
    bf16 = mybir.dt.bfloat16

    xw_d = nc.dram_tensor("xw", [BPC, S, D], fp16, kind="ExternalInput").ap()
    # cst packs the score bias columns (KC) and the reciprocal-denominator
    # columns (MC) into one small per-batch DMA.
    cst_d = nc.dram_tensor("cst", [BPC, P, KC + MC], f32,
                           kind="ExternalInput").ap()
    out_d = nc.dram_tensor("out", [BPC, T, D], bf16, kind="ExternalOutput").ap()

    rsv = 1.0 / math.sqrt(VARIANCE)
    AF = mybir.ActivationFunctionType

    # score pieces (k, t0, t1) in frame order; matmul chunk lists per m
    pieces2, mk2 = [], []
    for bands in bands2:
        pieces = []
        for k, band in enumerate(bands):
            if band is None:
                continue
            a, b = band
            # Cut at the absolute iota milestones so no piece waits on a
            # later trow chunk than it has to (GpSimd produces trow in
            # [.., 2048), [2048, 3072), [3072, 4096) order).
            marks = [c for c in (2048, 3072) if a < c < b]
            for lo, hi in zip([a] + marks, marks + [b]):
                t0 = lo
                while t0 < hi:
                    t1 = min(t0 + ACT_PIECE, hi)
                    pieces.append((k, t0, t1))
                    t0 = t1
        pieces.sort(key=lambda p: (p[1], p[0]))
        if pieces and pieces[0][2] - pieces[0][1] > 1024:
            k, t0, t1 = pieces[0]
            pieces[0:1] = [(k, t0, t0 + 256), (k, t0 + 256, t0 + 512),
                           (k, t0 + 512, t0 + 1024), (k, t0 + 1024, t1)]
        pieces2.append(pieces)
        mk = []
        for m in range(MC):
            ks = [k for k, band in enumerate(bands)
                  if band and m * P < band[1] and (m + 1) * P > band[0]]
            assert ks, f"no active token chunk for m={m}"
            mk.append(ks)
        mk2.append(mk)

    # Output groups of 4 consecutive frame-chunks per batch (uniform
    # groups keep the DMA flow smooth; larger ones bunch the wire and
    # stall the mul engines). Batch 0 leads while batch 1's scores are
    # still being produced, then the two batches interleave so the
    # engines see no cliff.
    GSIZES = [4] * 7 + [2, 2]
    assert sum(GSIZES) == MC

    def batch_groups(b):
        out, m0 = [], 0
        for gs in GSIZES:
            out.append((b, list(range(m0, m0 + gs))))
            m0 += gs
        return out

    g0, g1 = batch_groups(0), batch_groups(1)
    lead = 3
    group_seq = list(g0[:lead])
    rest0 = g0[lead:]
    i0 = i1 = 0
    while i0 < len(rest0) or i1 < len(g1):
        if i0 < len(rest0):
            group_seq.append(rest0[i0]); i0 += 1
        if i1 < len(g1):
            group_seq.append(g1[i1]); i1 += 1

    # Normalization engine per output chunk: early chunks stay on DVE
    # (ACT is still producing scores); later chunks split so ACT ends up
    # with ~24 of the 64 muls and DVE with the rest (GpSimd cannot read
    # PSUM, so it can't help here).
    N_EARLY, N_ACT = 16, 24
    late = 64 - N_EARLY
    engines = ["dve"] * N_EARLY + [
        "act" if (i * N_ACT) // late != ((i - 1) * N_ACT) // late else "dve"
        for i in range(1, late + 1)
    ]

    with tile.TileContext(nc) as tc:
        with tc.tile_pool(name="const", bufs=1) as constp, \
             tc.tile_pool(name="sb", bufs=2) as sb, \
             tc.tile_pool(name="outp", bufs=4) as outp, \
             tc.tile_pool(name="colp", bufs=4) as colp, \
             tc.tile_pool(name="ps", bufs=4, space="PSUM") as ps:

            # Warm the PE HAM clock gate: junk matmuls while the real
            # inputs are still loading, so real matmuls run at 2.4GHz.
            # The first activation on the zeroed junk tile also pulls in
            # the erf_derivative ACT table before any real score work.
            junk = constp.tile([P, 512], fp16)
            nc.gpsimd.memset(junk[:], 0.0)
            warm = colp.tile([P, 1], f32, name="warm", tag="warm", bufs=1)
            nc.scalar.activation(warm[:], junk[:, 0:1], AF.Derivative_Erf)
            for _ in range(6):
                jp = ps.tile([P, 512], f32, name="jp", tag="pt")
                nc.tensor.matmul(jp[:], junk[:, 0:P], junk[:],
                                 start=True, stop=True)

            # trow (frame indices 1..T) is generated on the idle GpSimd
            # engine in pieces, keeping the DMA wire free for real inputs.
            trow = constp.tile([P, T], f32)
            iota_cuts = [0, 256, 512, 1024, 2048, 3072, 4096]
            for q0, q1 in zip(iota_cuts, iota_cuts[1:]):
                nc.gpsimd.iota(trow[:, q0:q1],
                               pattern=[[1, q1 - q0]], base=1 + q0,
                               channel_multiplier=0,
                               allow_small_or_imprecise_dtypes=True)

            # All input DMAs up front on the Sync queue, before any output
            # issue can block them (the queue drains in program order).
            # xw is split in halves so the first matmuls can start as soon
            # as chunks 0-1 land; batch 0's first half jumps the queue.
            csts = []
            for b in range(BPC):
                cst = colp.tile([P, KC + MC], f32, name="cst", tag="cst")
                csts.append(cst)
            xws = []
            for b in range(BPC):
                xw = sb.tile([P, KC, D], fp16, name="xw_t", tag="xw_t")
                xws.append(xw)
            # Chunks whose band is None are never read by any matmul, so
            # their xw slices are never loaded (slot 0's k3 is fully
            # masked for every slot-0 batch).
            kmax = [max(k for k, band in enumerate(bands) if band) + 1
                    for bands in bands2]
            nc.sync.dma_start(out=csts[0][:], in_=cst_d[0])
            xw_src0 = xw_d[0].rearrange("(k p) d -> p k d", p=P)
            xw_src1 = xw_d[1].rearrange("(k p) d -> p k d", p=P)
            # b0's k0 chunk alone unlocks the first two output groups.
            nc.sync.dma_start(out=xws[0][:, 0:1, :], in_=xw_src0[:, 0:1, :])
            nc.sync.dma_start(out=csts[1][:], in_=cst_d[1])
            nc.sync.dma_start(out=xws[0][:, 1:2, :], in_=xw_src0[:, 1:2, :])
            if kmax[0] > 2:
                nc.sync.dma_start(out=xws[0][:, 2:kmax[0], :],
                                  in_=xw_src0[:, 2:kmax[0], :])
            nc.sync.dma_start(out=xws[1][:, 0:2, :], in_=xw_src1[:, 0:2, :])
            if kmax[1] > 2:
                nc.sync.dma_start(out=xws[1][:, 2:kmax[1], :],
                                  in_=xw_src1[:, 2:kmax[1], :])

            # All score production first (ACT stream order), so batch 1's
            # scores don't queue behind batch 0's ACT-side normalizations.
            # One pass: Derivative_Erf((t - c) / sqrt(V)) is a Gaussian up
            # to a constant factor folded into the host-side rcl.
            score_tiles = []
            for b in range(BPC):
                cst = csts[b]
                scores = sb.tile([P, KC, T], fp16, name="scores", tag="scores")
                for k, t0, t1 in pieces2[b]:
                    nc.scalar.activation(
                        scores[:, k, t0:t1], trow[:, t0:t1], AF.Derivative_Erf,
                        bias=cst[:, k:k + 1], scale=rsv,
                    )
                score_tiles.append(scores)

            chunk_idx = 0
            for seq_idx, (b, ms) in enumerate(group_seq):
                xw = xws[b]
                cst = csts[b]
                scores = score_tiles[b]
                ot = outp.tile([P, len(ms), D], bf16, name="ot", tag="ot")
                for g, m in enumerate(ms):
                    ks = mk2[b][m]
                    pt = ps.tile([P, D], f32, name="pt", tag="pt")
                    for i, k in enumerate(ks):
                        lhsT = scores[:, k, m * P:(m + 1) * P]
                        st = (i == 0)
                        sp = (i == len(ks) - 1)
                        mma = nc.tensor.matmul(
                            pt[:, 0:N0], lhsT, xw[:, k, 0:N0],
                            start=st, stop=sp,
                        )
                        mmb = nc.tensor.matmul(
                            pt[:, N0:D], lhsT, xw[:, k, N0:D],
                            start=st, stop=sp,
                        )
                        add_dep_helper(mmb.ins, mma.ins,
                                       reason="keep N-pieces adjacent")
                    if engines[chunk_idx] == "act":
                        nc.scalar.activation(
                            ot[:, g, :], pt[:], AF.Copy,
                            scale=cst[:, KC + m:KC + m + 1],
                        )
                    else:
                        nc.vector.tensor_scalar_mul(
                            ot[:, g, :], pt[:], cst[:, KC + m:KC + m + 1]
                        )
                    chunk_idx += 1
                nc.sync.dma_start(
                    out=out_d[b, ms[0] * P:(ms[-1] + 1) * P, :]
                    .rearrange("(g p) d -> p g d", p=P),
                    in_=ot[:],
                )

    nc.compile()
    return nc


def _get_program(bands):
    prog = _PROGRAMS.get(bands)
    if prog is None:
        prog = _build_program(bands)
        _PROGRAMS[bands] = prog
    return prog


def _prepare(x, d, mask):
    x = np.asarray(x, dtype=np.float32)
    d64 = np.asarray(d, dtype=np.float64)
    mask = np.asarray(mask, dtype=bool)

    e = np.cumsum(d64, axis=-1)
    c = e - 0.5 * d64                      # (B, S) token centers
    c_m = np.where(mask, c, np.nan)

    # Sort batches by valid length; slot 0 takes the 8 shortest, slot 1 the
    # 8 longest. Similar lengths per slot give much tighter per-slot bands.
    order = np.argsort(mask.sum(1), kind="stable")
    bands2 = tuple(
        _compute_bands(c_m[order[s * NCORES:(s + 1) * NCORES]])
        for s in range(BPC)
    )

    # Softmax denominators from d alone (float64, banded: contributions
    # beyond +-24 frames are < 1e-25 relative). rcl folds the table's
    # 2/sqrt(pi) factor.
    t_idx = np.arange(1, T + 1, dtype=np.float64)
    rcl = np.empty((B, P, MC), dtype=np.float32)
    for b in range(B):
        cb = c[b][mask[b]]
        Db = np.zeros(T, dtype=np.float64)
        base = np.round(cb).astype(np.int64) - 1
        for off in range(-24, 25):
            j = base + off
            ok = (j >= 0) & (j < T)
            np.add.at(Db, j[ok],
                      np.exp(-((t_idx[j[ok]] - cb[ok]) ** 2) / VARIANCE))
        r = (math.sqrt(math.pi) / 2.0) / Db
        rcl[b] = r.reshape(MC, P).T.astype(np.float32)

    # Masked tokens: keep the table input bounded (|x| <= ~2250) and rely
    # on their zeroed xw rows to make any residual score value inert.
    c = np.where(mask, c, -3000.0)
    bias = (-c / math.sqrt(VARIANCE)).astype(np.float32)

    # cst packs [P, KC] bias columns and [P, MC] reciprocal columns.
    cst = np.empty((B, P, KC + MC), dtype=np.float32)
    cst[:, :, :KC] = bias.reshape(B, KC, P).transpose(0, 2, 1)
    cst[:, :, KC:] = rcl

    xw = np.where(mask[:, :, None], x, 0.0).astype(np.float16)

    in_maps = []
    for core in range(NCORES):
        idx = [order[core], order[NCORES + core]]
        in_maps.append({
            "xw": np.ascontiguousarray(xw[idx]),
            "cst": np.ascontiguousarray(cst[idx]),
        })
    return in_maps, bands2, order


def run(x, d, mask, frame_length, trace=False):
    assert int(frame_length) == T
    in_maps, bands2, order = _prepare(x, d, mask)
    nc = _get_program(bands2)
    res = None
    for attempt in range(3):
        try:
            res = run_bass_kernel_spmd(nc, in_maps, list(range(NCORES)),
                                       trace=trace)
            break
        except Exception:
            # The first execution after a fresh compile occasionally hits a
            # transient device error; retrying succeeds.
            if attempt == 2:
                raise
    out = np.empty((B, T, D), dtype=np.float32)
    for core in range(NCORES):
        for s in range(BPC):
            out[order[s * NCORES + core]] = (
                res.results[core]["out"][s].astype(np.float32)
            )
    return out, res


def kernel(x, d, mask, frame_length):
    out, _ = run(x, d, mask, frame_length, trace=False)
    return out
